# revision 22
# baseline (speedup 1.0000x reference)
"""Distributed KNN (analogy-based estimation) kernel for 8 TRN2 NeuronCores.

Strategy v2 (candidate-major scan + PE cell-reduction):
  - Shard the train set (N=65536) across 8 cores (8192 rows each); replicate
    the 2048 queries.  All tensors fit in SBUF.
  - Phase 1: fp8 DoubleRow matmuls with stationary = train block (128
    candidates, K=256) and moving = 512 queries -> PSUM scores in
    CANDIDATE-major layout [128 cand, 512 q].
  - Relu evacuation: ScalarE (activation Relu, bias -T) and VectorE
    (tensor_scalar add/max) split the flat relu(s-T) -> fp8 SBUF pass.
    This is the only per-score work outside the PE: no GpSimd folds, no
    DVE reduces (those were the old bottleneck at ~86% engine busy).
  - Phase 2: the 32-candidate cell reduction runs on the PE itself: a
    block-ones stationary [128,2,32] (M=32 -> cells of 8 = 4 rows x 2
    blocks) contracts the relu'd fp8 scores; 4 consecutive pairs
    accumulate into the same PSUM cells -> cells of 32 candidates spread
    over 8 contiguous 4-row units.  Matmul output base partitions are
    restricted to {0,32,64}, so 3 accumulation groups pack per PSUM bank
    and 6 per 2-bank stat tile ([96,1024] evacuated in one instruction).
  - Stats (bf16) DMA out in device layout; host reshapes to
    [core, query, 256 cells], takes top-16 cells per (row, core), expands
    to 4-row candidate blocks, then the same coarse-f32 / exact-f64
    refinement + label epilogue as v1.
"""

from contextlib import ExitStack

import numpy as np
import ml_dtypes

import concourse.bass as bass
import concourse.mybir as mybir
import concourse.tile as tile
from concourse import bacc
from concourse.bass_utils import run_bass_kernel_spmd

N_CORES = 8
B = 2048          # queries
N_TRAIN = 65536   # train rows
F = 256           # features
NSHARD = N_TRAIN // N_CORES   # 8192 train rows per core

QG = 512                      # queries per moving group
N_QG = B // QG                # 4
BLK = 128                     # candidates per phase-1 matmul (stationary M)
N_BLK = NSHARD // BLK         # 64 blocks per core
N_PAIR = N_BLK // 2           # 32 pairs per core (per qg reuse)
ACC = 8                       # pairs accumulated into one phase-2 group
N_GRP = N_PAIR // ACC         # 4 accumulation groups per qg
CELL = 32                     # candidates per cell (2 rows x 16 blocks)
N_CELLS = NSHARD // CELL      # 256 cells per (query, core)
TOPC = 16                     # cells kept per (row, core) on host
RELU_T = 2.5
FP8_SCALE = 32.0

# relu-evacuation engine split: ACT_NUM of every ACT_DEN pairs go to ScalarE
# (the rest to VectorE), spread evenly.  DVE per-tile ~1192ns vs Act ~1038ns.
ACT_NUM = 17
ACT_DEN = 32
# software-pipeline lag (in pairs) between phase-1+relu and phase-2, so the
# in-order PE queue never stalls on a relu that hasn't run yet
PH2_LAG = 6

_BF16 = mybir.dt.bfloat16
_F32 = mybir.dt.float32
_FP8 = mybir.dt.float8e4


def _build(loop_reps=None):
    nc = bacc.Bacc("TRN2", target_bir_lowering=False, debug=False)
    xT = nc.dram_tensor("xT", [F, B], _FP8, kind="ExternalInput")
    tT = nc.dram_tensor("tT", [F, NSHARD], _FP8, kind="ExternalInput")
    onesW = nc.dram_tensor("onesW", [128, 128], _FP8, kind="ExternalInput")
    # device stat layout: [qg, tile(2), part(64), free(1024)] bf16
    out_cm = nc.dram_tensor(
        "cm_dev", [N_QG, 4, 64, 512], _BF16, kind="ExternalOutput"
    )

    with tile.TileContext(nc) as tc, ExitStack() as ctx:
        const = ctx.enter_context(tc.tile_pool(name="const", bufs=1))
        ps1p = ctx.enter_context(tc.tile_pool(name="ps1", bufs=3, space="PSUM"))
        ps2p = ctx.enter_context(tc.tile_pool(name="ps2", bufs=2, space="PSUM"))
        scrp = ctx.enter_context(tc.tile_pool(name="scr", bufs=2))
        statp = ctx.enter_context(tc.tile_pool(name="stat", bufs=3))

        x_all = const.tile([128, 2 * B], _FP8, name="x_all")
        t_all = const.tile([128, 2 * NSHARD], _FP8, name="t_all")
        ones_sb = const.tile([128, 128], _FP8, name="ones_sb")

        # chunked loads so phase-1 can start before the full inputs land
        XCH = 4
        for ch in range(XCH):
            for f in range(2):
                w = B // XCH
                nc.sync.dma_start(
                    x_all[:, f * B + ch * w:f * B + (ch + 1) * w],
                    xT[f * 128:(f + 1) * 128, ch * w:(ch + 1) * w],
                )
        TCH = 8
        for ch in range(TCH):
            for f in range(2):
                w = NSHARD // TCH
                nc.sync.dma_start(
                    t_all[:, f * NSHARD + ch * w:f * NSHARD + (ch + 1) * w],
                    tT[f * 128:(f + 1) * 128, ch * w:(ch + 1) * w],
                )
        nc.sync.dma_start(ones_sb[:], onesW[:])

        neg_t = const.tile([128, 1], _F32, name="neg_t")
        nc.vector.memset(neg_t[:], -RELU_T * FP8_SCALE)

        x_dr = x_all[:].rearrange("p (i w) -> p i w", i=2)
        t_dr = t_all[:].rearrange("p (i w) -> p i w", i=2)
        ones_v = ones_sb[:].rearrange("p (i m) -> p i m", i=2)

        def compute():
            _compute(nc, tc, x_dr, t_dr, ones_v, neg_t, ps1p, ps2p, scrp,
                     statp, out_cm)

        if loop_reps is not None:
            with tc.For_i(0, loop_reps, 1):
                compute()
        else:
            compute()
    nc.compile()
    return nc


def _compute(nc, tc, x_dr, t_dr, ones_v, neg_t, ps1p, ps2p, scrp, statp,
             out_cm):
    dr = mybir.MatmulPerfMode.DoubleRow
    TOT = N_QG * N_PAIR            # 128 pairs across the whole compute
    scratches = {}
    ps2_tiles = {}

    def emit_ph1(idx):
        qg, p = divmod(idx, N_PAIR)
        if p == 0:
            scratches[qg] = scrp.tile(
                [128, N_PAIR * 1024], _FP8, tag="scr", name=f"scr_{qg}"
            )
        xg = x_dr[:, :, qg * QG:(qg + 1) * QG]
        ps = ps1p.tile([128, 1024], _F32, tag="ps", name=f"ps_{idx}")
        for jj in range(2):
            b = 2 * p + jj
            nc.tensor.matmul(
                ps[:, jj * 512:(jj + 1) * 512],
                t_dr[:, :, b * BLK:(b + 1) * BLK],
                xg,
                start=True, stop=True,
                perf_mode=dr,
            )
        sl = scratches[qg][:, p * 1024:(p + 1) * 1024]
        # Bresenham-spread Act/DVE split, ACT_NUM of every ACT_DEN on ScalarE
        if (idx * ACT_NUM) % ACT_DEN < ACT_NUM:
            nc.scalar.activation(
                sl, ps[:], mybir.ActivationFunctionType.Relu, bias=neg_t[:]
            )
        else:
            nc.vector.tensor_scalar(
                sl, ps[:], -RELU_T * FP8_SCALE, 0.0,
                op0=mybir.AluOpType.add, op1=mybir.AluOpType.max,
            )

    def emit_ph2(idx):
        qg, p = divmod(idx, N_PAIR)
        g2 = p // ACC                  # accumulation group = one ps2 bank
        if p % ACC == 0:
            ps2_tiles[(qg, g2)] = ps2p.tile(
                [128, 512], _F32, tag="ps2", name=f"ps2_{qg}_{g2}"
            )
        ps2 = ps2_tiles[(qg, g2)]
        out_sl = ps2[0:64, :]
        sl = scratches[qg][:, p * 1024:(p + 1) * 1024]
        nc.tensor.matmul(
            out_sl,
            ones_v,
            sl.rearrange("q (i w) -> q i w", i=2),
            start=(p % ACC == 0), stop=(p % ACC == ACC - 1),
            perf_mode=dr,
            skip_group_check=True,
        )
        if p % ACC == ACC - 1:         # group complete -> evacuate stats
            st = statp.tile([64, 512], _BF16, tag="st",
                            name=f"st_{qg}_{g2}")
            if (qg * 4 + g2) % 2 == 0:
                nc.scalar.activation(
                    st, ps2[0:64, :], mybir.ActivationFunctionType.Copy
                )
            else:
                nc.vector.tensor_copy(st, ps2[0:64, :])
            nc.sync.dma_start(out_cm[qg, g2, :, :], st)

    for idx in range(TOT + PH2_LAG):
        if idx < TOT:
            emit_ph1(idx)
        if idx >= PH2_LAG:
            emit_ph2(idx - PH2_LAG)


UNITS_PER_CELL = 2 * ACC      # 16 two-row units per cell
UNIT = CELL // UNITS_PER_CELL  # 2 rows per unit


def _cells_to_blocks(cid):
    """cell id -> the 16 two-row train blocks it covers (within-shard).

    cell c: accumulation group g2 = c//64 covers pairs [8*g2, 8*g2+8)
    = 128-row blocks [16*g2, 16*g2+16); sub-row m = c%64 selects rows
    [2m, 2m+2) of each block.  2-row unit id = block128 * 64 + m.
    """
    g2 = cid // 64
    m = cid % 64
    b128 = 16 * g2[..., None] + np.arange(16, dtype=np.int64)
    return b128 * 64 + m[..., None]          # [..., 16] 2-row unit ids


_CACHE = {}


def _prep_inputs(x_input, train_inputs):
    x = np.asarray(x_input, np.float32)
    xh = x / (np.linalg.norm(x, axis=1, keepdims=True) + 1e-30)
    xh = xh * FP8_SCALE
    in_np_dt = ml_dtypes.float8_e4m3
    xT = np.ascontiguousarray(xh.T).astype(in_np_dt)
    ones = np.zeros((128, 128), dtype=in_np_dt)
    for m in range(64):
        ones[2 * m:2 * m + 2, m] = 1.0        # i=0 half
        ones[2 * m:2 * m + 2, 64 + m] = 1.0   # i=1 half
    in_maps = []
    for s in range(N_CORES):
        shard = np.asarray(
            train_inputs[s * NSHARD:(s + 1) * NSHARD], np.float32
        )
        tTs = np.ascontiguousarray(shard.T).astype(in_np_dt)
        in_maps.append({"xT": xT, "tT": tTs, "onesW": ones})
    return in_maps


def _run_device(x_input, train_inputs, trace=False, **kw):
    if "nc" not in _CACHE:
        _CACHE["nc"] = _build()
    nc = _CACHE["nc"]
    in_maps = _prep_inputs(x_input, train_inputs)
    return run_bass_kernel_spmd(
        nc, in_maps, core_ids=list(range(N_CORES)), trace=trace, **kw
    )


def _stats_to_cm(dev):
    """Device stat tensor [N_QG, 4, 64, 512] bf16 -> [B, N_CELLS] f32.

    dev[qg, g2, m, ql] = stat of cell g2*64 + m for query qg*512 + ql.
    """
    dev = np.asarray(dev).astype(np.float32)
    v = dev.transpose(0, 3, 1, 2)                  # [qg, ql, g2, m]
    return np.ascontiguousarray(v.reshape(B, N_CELLS))


def kernel(x_input, train_inputs, features, train_labels, num_k, num_labels):
    x = np.asarray(x_input, dtype=np.float32)
    train = np.asarray(train_inputs, dtype=np.float32)
    feats = np.asarray(features, dtype=np.float32)
    labels = np.asarray(train_labels)
    k = int(num_k)
    L = int(num_labels)

    res = _run_device(x, train)
    cm = np.stack(
        [_stats_to_cm(res.results[s]["cm_dev"]) for s in range(N_CORES)],
        axis=0,
    )  # [cores, B, N_CELLS] f32 cell statistics

    # Host-side selection: top-TOPC cells per (core, row) by statistic.
    flat = cm.reshape(-1, N_CELLS)
    part = np.argpartition(-flat, TOPC - 1, axis=1)[:, :TOPC]
    cid = part.reshape(N_CORES, B, TOPC).astype(np.int64)

    # Expand top cells to candidate BLOCKS of 2 contiguous train rows.
    blk = _cells_to_blocks(cid)                        # [cores, B, TOPC, 16]
    blk = blk + (np.arange(N_CORES, dtype=np.int64) * (NSHARD // UNIT))[
        :, None, None, None
    ]
    blk = blk.transpose(1, 0, 2, 3).reshape(B, -1)     # [B, cores*TOPC*16]
    blk = np.sort(blk, axis=1)
    NBLK = blk.shape[1]                                # 2048
    dupb = np.zeros(blk.shape, dtype=bool)
    dupb[:, 1:] = blk[:, 1:] == blk[:, :-1]

    # Refinement: coarse f32 pass narrows ~4k candidates/row to 8, then an
    # exact float64 pass ranks those with the reference's tie-breaking.
    w = feats[None, :] * train
    right32 = np.einsum("nf,nf->n", w, w, dtype=np.float32)
    left32 = np.einsum("bf,bf->b", x, x, dtype=np.float32)
    w64 = w.astype(np.float64)
    x64 = x.astype(np.float64)
    left64 = np.einsum("bf,bf->b", x64, x64)

    train_blocks = train.reshape(N_TRAIN // UNIT, UNIT * F)
    NARROW = 8
    topk_idx = np.empty((B, k), dtype=np.int64)
    CH = 128
    gbuf = np.empty((CH * NBLK, UNIT * F), dtype=np.float32)
    for r0 in range(0, B, CH):
        r1 = min(B, r0 + CH)
        bi = blk[r0:r1]                                # [rows, NBLK]
        ci = (bi[:, :, None] * UNIT + np.arange(UNIT)).reshape(r1 - r0, -1)
        np.take(train_blocks, bi.ravel(), axis=0, out=gbuf)
        tcand = gbuf.reshape(r1 - r0, NBLK * UNIT, F)  # [rows, nc, F]
        cross = np.matmul(tcand, x[r0:r1][:, :, None])[..., 0]
        d32 = np.sqrt(left32[r0:r1, None] + right32[ci]) - 2.0 * cross
        d32.reshape(r1 - r0, NBLK, UNIT)[dupb[r0:r1]] = np.inf
        part = np.argpartition(d32, NARROW, axis=1)[:, :NARROW]
        ci8 = np.take_along_axis(ci, part, axis=1)     # [rows, 8] distinct
        ci8.sort(axis=1)
        # exact f64 distances for the 8 finalists
        t8 = train[ci8].astype(np.float64)
        cross8 = np.matmul(t8, x64[r0:r1][:, :, None])[..., 0]
        w8 = w64[ci8]
        r8 = np.einsum("bkf,bkf->bk", w8, w8)
        d8 = np.sqrt(left64[r0:r1, None] + r8) - 2.0 * cross8
        dup8 = np.zeros(ci8.shape, dtype=bool)
        dup8[:, 1:] = ci8[:, 1:] == ci8[:, :-1]
        d8[dup8] = np.inf
        order = np.argsort(d8, axis=1, kind="stable")[:, :k]
        topk_idx[r0:r1] = np.take_along_axis(ci8, order, axis=1)

    lab = labels[topk_idx]               # [B, k] (int64)
    lab_kb = lab.reshape(k, B)           # faithful [B,k] -> [k,B] reshape
    outputs = lab_kb.sum(axis=0) // k
    out = np.zeros((B, L), dtype=np.float32)
    out[np.arange(B), outputs] = 1.0
    return out


# revision 34
# speedup vs baseline: 1.0739x; 1.0739x over previous
"""Distributed KNN (analogy-based estimation) kernel for 8 TRN2 NeuronCores.

Strategy v3 (query-major scan, three-engine balanced evacuation):
  - Shard the train set (N=65536) across 8 cores (8192 rows each); replicate
    the 2048 queries.  Queries are L2-normalized on the host and fp8-scaled
    so a global relu threshold is calibrated; the cross term alone ranks
    candidates (norm terms only perturb by O(1) while top-of-65536 gaps are
    O(10) after scaling).
  - Phase 1 (PE): fp8 DoubleRowSwInterleave matmuls, stationary = query
    tile (host-interleaved so the 256-col weight load reads contiguously,
    ~249 ns/MM measured vs 309 ns plain DoubleRow), moving = train chunks
    [p,2,512] -> PSUM scores [128 q, 1024 cand] per tile, 8 tiles/q-tile.
  - Evacuation, balanced across the three non-PE engines (HW-measured
    rates: Act relu [128,1024] ~1.1us, DVE direct reduce ~1.25us, GpSimd
    reduce ~2.3us):
      * DIRECT tiles: DVE tensor_reduce max over 32-candidate cells
        straight from PSUM (raw max; host shifts by -T*scale).
      * other tiles: ScalarE relu(s - T) -> bf16 scratch, then a 32-wide
        add-reduce on GpSimd or DVE (Bresenham-spread) -> cell sums.
  - Cell stats land in a [128, 256] bf16 tile per q-tile, DMA'd out.
  - Host: top-16 cells per (row, core), expand to two 16-row blocks per
    cell, coarse f32 distance pass narrows ~4k candidates/row to 8, exact
    f64 pass ranks them with the reference's tie-breaking, then the label
    gather / faithful [B,k]->[k,B] reshape / integer-mean / one-hot
    epilogue in exact integer arithmetic.
"""

from contextlib import ExitStack

import numpy as np
import ml_dtypes

import concourse.bass as bass
import concourse.mybir as mybir
import concourse.tile as tile
from concourse import bacc
from concourse.bass_utils import run_bass_kernel_spmd

N_CORES = 8
B = 2048          # queries
N_TRAIN = 65536   # train rows
F = 256           # features
NSHARD = N_TRAIN // N_CORES   # 8192 train rows per core

Q_TILE = 128
N_QT = B // Q_TILE            # 16 query tiles
CHUNK = 512                   # matmul moving free dim (one PSUM bank f32)
TILE_W = 1024                 # psum tile width (2 banks, 2 chunks)
N_PT = NSHARD // TILE_W       # 8 psum tiles per (q-tile, core)
CELL = 32                     # candidates per cell (contiguous)
CELLS_PER_TILE = TILE_W // CELL   # 32
N_CELLS = N_PT * CELLS_PER_TILE   # 256 cells per (row, core)
TOPC = 16                     # cells kept per (row, core) on host
RELU_T = 2.5
FP8_SCALE = 32.0

# evacuation mix per q-tile (q-alternating so DVE/Pool balance at the
# fractional optimum d=2.5):
#   direct tiles: DVE max-reduce straight from PSUM (raw max stat)
#   all others:   ScalarE relu -> GpSimd half-fold -> DVE 16-wide reduce
def _direct_set(q):
    return (1, 4, 6) if q % 2 == 0 else (2, 5)

_BF16 = mybir.dt.bfloat16
_F32 = mybir.dt.float32
_FP8 = mybir.dt.float8e4

# timing probe: None = real kernel; "pe" = phase-1 matmuls only
PROBE = None
LDW_SWI = True


def _build(loop_reps=None):
    nc = bacc.Bacc("TRN2", target_bir_lowering=False, debug=False)
    x_shape = [128, 2 * B] if LDW_SWI else [F, B]
    xT = nc.dram_tensor("xT", x_shape, _FP8, kind="ExternalInput")
    tT = nc.dram_tensor("tT", [F, NSHARD], _FP8, kind="ExternalInput")
    out_cm = nc.dram_tensor("cm_dev", [B, N_CELLS], _BF16,
                            kind="ExternalOutput")

    with tile.TileContext(nc) as tc, ExitStack() as ctx:
        const = ctx.enter_context(tc.tile_pool(name="const", bufs=1))
        psp = ctx.enter_context(tc.tile_pool(name="ps", bufs=4, space="PSUM"))
        scrp = ctx.enter_context(tc.tile_pool(name="scr", bufs=3))
        cmaxp = ctx.enter_context(tc.tile_pool(name="cmax", bufs=2))

        x_all = const.tile([128, 2 * B], _FP8, name="x_all")
        t_all = const.tile([128, 2 * NSHARD], _FP8, name="t_all")

        if LDW_SWI:
            XCH = 4
            for ch in range(XCH):
                w = 2 * B // XCH
                nc.sync.dma_start(
                    x_all[:, ch * w:(ch + 1) * w], xT[:, ch * w:(ch + 1) * w]
                )
        else:
            for f in range(2):
                nc.sync.dma_start(
                    x_all[:, f * B:(f + 1) * B], xT[f * 128:(f + 1) * 128, :]
                )
        TCH = 8
        for ch in range(TCH):
            for f in range(2):
                w = NSHARD // TCH
                nc.sync.dma_start(
                    t_all[:, f * NSHARD + ch * w:f * NSHARD + (ch + 1) * w],
                    tT[f * 128:(f + 1) * 128, ch * w:(ch + 1) * w],
                )

        neg_t = const.tile([128, 1], _F32, name="neg_t")
        nc.vector.memset(neg_t[:], -RELU_T * FP8_SCALE)

        t_dr = t_all[:].rearrange("p (i w) -> p i w", i=2)
        if LDW_SWI:
            x_sta = [x_all[:, q * 2 * Q_TILE:(q + 1) * 2 * Q_TILE]
                     for q in range(N_QT)]
        else:
            x_dr = x_all[:].rearrange("p (i w) -> p i w", i=2)
            x_sta = [x_dr[:, :, q * Q_TILE:(q + 1) * Q_TILE]
                     for q in range(N_QT)]

        def compute():
            _compute(nc, tc, x_sta, t_dr, neg_t, psp, scrp, cmaxp, out_cm)

        if loop_reps is not None:
            with tc.For_i(0, loop_reps, 1):
                compute()
        else:
            compute()
    nc.compile()
    return nc


def _compute(nc, tc, x_sta, t_dr, neg_t, psp, scrp, cmaxp, out_cm):
    pm = (mybir.MatmulPerfMode.DoubleRowSwInterleave if LDW_SWI
          else mybir.MatmulPerfMode.DoubleRow)

    for q in range(N_QT):
        if PROBE == "pe":
            for t in range(N_PT):
                ps = psp.tile([128, TILE_W], _F32, tag="ps", name=f"ps_{q}_{t}")
                for cc in range(2):
                    c = 2 * t + cc
                    nc.tensor.matmul(
                        ps[:, cc * CHUNK:(cc + 1) * CHUNK],
                        x_sta[q],
                        t_dr[:, :, c * CHUNK:(c + 1) * CHUNK],
                        start=True, stop=True, perf_mode=pm,
                    )
            continue
        cmax = cmaxp.tile([128, N_CELLS], _BF16, tag="cm", name=f"cm_{q}")
        for t in range(N_PT):
            ps = psp.tile([128, TILE_W], _F32, tag="ps", name=f"ps_{q}_{t}")
            for cc in range(2):
                c = 2 * t + cc
                nc.tensor.matmul(
                    ps[:, cc * CHUNK:(cc + 1) * CHUNK],
                    x_sta[q],
                    t_dr[:, :, c * CHUNK:(c + 1) * CHUNK],
                    start=True, stop=True, perf_mode=pm,
                )
            cm_out = cmax[:, t * CELLS_PER_TILE:(t + 1) * CELLS_PER_TILE]
            if t in _direct_set(q):
                # DVE evacuates+reduces straight from PSUM (raw cell max;
                # host shifts these columns by -T*scale before selection)
                nc.vector.tensor_reduce(
                    out=cm_out,
                    in_=ps[:].rearrange("p (c e) -> p c e", e=CELL),
                    axis=mybir.AxisListType.X,
                    op=mybir.AluOpType.max,
                )
            else:
                st = scrp.tile([128, TILE_W], _BF16, tag="st",
                               name=f"st_{q}_{t}")
                nc.scalar.activation(
                    st[:], ps[:], mybir.ActivationFunctionType.Relu,
                    bias=neg_t[:],
                )
                # bf16 accumulation of <=32 small nonneg values: plenty of
                # precision for coarse cell ranking (host refines exactly)
                with nc.allow_low_precision(reason="cell-stat ranking only"):
                    # GpSimd folds the halves (pairs candidates
                    # {16k, 16k+512}), DVE finishes with a 16-wide sum
                    gp = scrp.tile([128, TILE_W // 2], _BF16, tag="gp",
                                   name=f"gp_{q}_{t}")
                    nc.gpsimd.tensor_add(
                        gp[:], st[:, 0:TILE_W // 2],
                        st[:, TILE_W // 2:TILE_W],
                    )
                    nc.vector.tensor_reduce(
                        out=cm_out,
                        in_=gp[:].rearrange("p (c e) -> p c e", e=16),
                        axis=mybir.AxisListType.X,
                        op=mybir.AluOpType.add,
                    )
        qs = slice(q * Q_TILE, (q + 1) * Q_TILE)
        nc.sync.dma_start(out_cm[qs, :], cmax[:])


def _tile_masks():
    """[N_QT, N_CELLS] bool masks: which cells are direct per q-tile."""
    direct = np.zeros((N_QT, N_CELLS), dtype=bool)
    t_of_cell = np.arange(N_CELLS) // CELLS_PER_TILE
    for q in range(N_QT):
        direct[q] = np.isin(t_of_cell, _direct_set(q))
    return direct


def _cells_to_blocks(cid, qt):
    """cell id -> its two 16-row train blocks (within-shard).

    Direct tiles: cell c covers contiguous candidates [32c, 32c+32) ->
    units {2c, 2c+1}.  Pool-folded tiles pair candidates {16k, 16k+512}
    within the tile -> units {64t+k, 64t+32+k}.  qt = q-tile per row.
    """
    t = cid // CELLS_PER_TILE
    k = cid % CELLS_PER_TILE
    dmask = _tile_masks()                    # [N_QT, N_CELLS]
    direct = dmask[qt, cid]
    blk0 = np.where(direct, 2 * cid, 64 * t + k)
    blk1 = np.where(direct, 2 * cid + 1, 64 * t + 32 + k)
    return np.stack([blk0, blk1], axis=-1)


def _host_adjust(cm):
    """Direct tiles report raw cell max; put them on the relu(.-T) scale."""
    shift = RELU_T * FP8_SCALE
    dmask = _tile_masks()                    # [N_QT, N_CELLS]
    row_mask = np.repeat(dmask, Q_TILE, axis=0)      # [B, N_CELLS]
    cm[:, row_mask] -= shift
    return cm


_CACHE = {}


def _prep_inputs(x_input, train_inputs):
    x = np.asarray(x_input, np.float32)
    xh = x / (np.linalg.norm(x, axis=1, keepdims=True) + 1e-30)
    xh = xh * FP8_SCALE
    in_np_dt = ml_dtypes.float8_e4m3
    xT = np.ascontiguousarray(xh.T).astype(in_np_dt)
    if LDW_SWI:
        # swi packing per q-tile: [A127 B127 A126 B126 ... A0 B0] per
        # partition, where A/B are the two 128-feature halves and columns
        # are reversed
        xa = xT[:128].reshape(128, N_QT, Q_TILE)[:, :, ::-1]
        xb = xT[128:].reshape(128, N_QT, Q_TILE)[:, :, ::-1]
        xT = np.ascontiguousarray(
            np.stack([xa, xb], axis=-1).reshape(128, 2 * B)
        )
    in_maps = []
    for s in range(N_CORES):
        shard = np.asarray(
            train_inputs[s * NSHARD:(s + 1) * NSHARD], np.float32
        )
        tTs = np.ascontiguousarray(shard.T).astype(in_np_dt)
        in_maps.append({"xT": xT, "tT": tTs})
    return in_maps


def _run_device(x_input, train_inputs, trace=False, **kw):
    if "nc" not in _CACHE:
        _CACHE["nc"] = _build()
    nc = _CACHE["nc"]
    in_maps = _prep_inputs(x_input, train_inputs)
    return run_bass_kernel_spmd(
        nc, in_maps, core_ids=list(range(N_CORES)), trace=trace, **kw
    )


def kernel(x_input, train_inputs, features, train_labels, num_k, num_labels):
    x = np.asarray(x_input, dtype=np.float32)
    train = np.asarray(train_inputs, dtype=np.float32)
    feats = np.asarray(features, dtype=np.float32)
    labels = np.asarray(train_labels)
    k = int(num_k)
    L = int(num_labels)

    res = _run_device(x, train)
    cm = np.stack(
        [np.asarray(res.results[s]["cm_dev"]).astype(np.float32)
         for s in range(N_CORES)],
        axis=0,
    )  # [cores, B, N_CELLS] cell statistics
    cm = _host_adjust(cm)

    # Host-side selection: top-TOPC cells per (core, row) by statistic.
    flat = cm.reshape(-1, N_CELLS)
    part = np.argpartition(-flat, TOPC - 1, axis=1)[:, :TOPC]
    cid = part.reshape(N_CORES, B, TOPC).astype(np.int64)

    # Expand top cells to candidate BLOCKS of 16 contiguous train rows.
    qt = (np.arange(B) // Q_TILE)[None, :, None]       # q-tile of each row
    qt = np.broadcast_to(qt, cid.shape)
    blk = _cells_to_blocks(cid, qt)                    # [cores, B, TOPC, 2]
    blk = blk + (np.arange(N_CORES, dtype=np.int64) * (NSHARD // 16))[
        :, None, None, None
    ]
    blk = blk.transpose(1, 0, 2, 3).reshape(B, -1)     # [B, cores*TOPC*2]
    blk = np.sort(blk, axis=1)
    NBLK = blk.shape[1]                                # 256
    dupb = np.zeros(blk.shape, dtype=bool)
    dupb[:, 1:] = blk[:, 1:] == blk[:, :-1]

    # Refinement: coarse f32 pass narrows ~4k candidates/row to 8, then an
    # exact float64 pass ranks those with the reference's tie-breaking.
    w = feats[None, :] * train
    right32 = np.einsum("nf,nf->n", w, w, dtype=np.float32)
    left32 = np.einsum("bf,bf->b", x, x, dtype=np.float32)
    w64 = w.astype(np.float64)
    x64 = x.astype(np.float64)
    left64 = np.einsum("bf,bf->b", x64, x64)

    train_blocks = train.reshape(N_TRAIN // 16, 16 * F)
    NARROW = 8
    topk_idx = np.empty((B, k), dtype=np.int64)
    CH = 128
    gbuf = np.empty((CH * NBLK, 16 * F), dtype=np.float32)
    for r0 in range(0, B, CH):
        r1 = min(B, r0 + CH)
        bi = blk[r0:r1]                                # [rows, NBLK]
        ci = (bi[:, :, None] * 16 + np.arange(16)).reshape(r1 - r0, -1)
        np.take(train_blocks, bi.ravel(), axis=0, out=gbuf)
        tcand = gbuf.reshape(r1 - r0, NBLK * 16, F)    # [rows, nc, F]
        cross = np.matmul(tcand, x[r0:r1][:, :, None])[..., 0]
        d32 = np.sqrt(left32[r0:r1, None] + right32[ci]) - 2.0 * cross
        d32.reshape(r1 - r0, NBLK, 16)[dupb[r0:r1]] = np.inf
        part = np.argpartition(d32, NARROW, axis=1)[:, :NARROW]
        ci8 = np.take_along_axis(ci, part, axis=1)     # [rows, 8] distinct
        ci8.sort(axis=1)
        # exact f64 distances for the 8 finalists
        t8 = train[ci8].astype(np.float64)
        cross8 = np.matmul(t8, x64[r0:r1][:, :, None])[..., 0]
        w8 = w64[ci8]
        r8 = np.einsum("bkf,bkf->bk", w8, w8)
        d8 = np.sqrt(left64[r0:r1, None] + r8) - 2.0 * cross8
        dup8 = np.zeros(ci8.shape, dtype=bool)
        dup8[:, 1:] = ci8[:, 1:] == ci8[:, :-1]
        d8[dup8] = np.inf
        order = np.argsort(d8, axis=1, kind="stable")[:, :k]
        topk_idx[r0:r1] = np.take_along_axis(ci8, order, axis=1)

    lab = labels[topk_idx]               # [B, k] (int64)
    lab_kb = lab.reshape(k, B)           # faithful [B,k] -> [k,B] reshape
    outputs = lab_kb.sum(axis=0) // k
    out = np.zeros((B, L), dtype=np.float32)
    out[np.arange(B), outputs] = 1.0
    return out
